# revision 19
# baseline (speedup 1.0000x reference)
"""AnalyticGaussianVelocity (soft-kNN flow velocity) on 8 trn2 NeuronCores.

Math (reference):
    a = t, b = 1-t
    logit[b,n] = -1/(2 b^2) * ||x_b - a * d_n||^2
    prob = softmax(logit, axis=n) * (1 + a/b)
    v = (-1/b) x + prob @ dataset

Dropping per-row constants, softmax(logit) == softmax(u * P) with
    u = a/b^2  (>0),  P[b,n] = x_b . d_n - (a/2) ||d_n||^2

Kernel strategy (dataset sharded over N across 8 cores, flash-style
online softmax per core, AllReduce merge):
  host prep: dataset transposed (dT fp32 [D,n]), bf16 copy (natb [n,D]),
       norms dn = ||d_n||^2 with 3-way bf16 splits packed as dn6 [6,n],
       w = -(a/2) 3-way bf16 split packed as w6 [6,B], xT fp32 [D,B].
  MM1: P = xT.T @ dT as a SINGLE float32r pass (hw-validated: f32r keeps
       11 explicit mantissa bits, 1 cyc/row; end-to-end err 1.6e-3)
       + a K=6 bf16 matmul folding in the -(a/2)||d||^2 term
       (w1*dn1+w1*dn2+w1*dn3+w2*dn1+w2*dn2+w3*dn1 ~ full fp32 product).
       f32r operands are rounded on device (compiler requires a rounding
       producer): dT chunks on Pool, xT at setup.
  softmax: DVE row-max -> ACT exp(scale=u, bias=-u*m) with free row-sum
       (accum_out), prob emitted in bf16.
  MM2: acc_new = diag(alpha) @ acc (f32r rescale matmul) + probT @ natb
       (bf16); probT via PE transposes + copy back to SBUF.
  merge: AllReduce-max of m, rescale by exp(u(m_loc-m_glob)),
         AllReduce-add of [acc | l], then v = dcoef*acc/l + vcoef*x.
"""

import sys

sys.path.insert(0, "/opt/trn_rl_repo")

import numpy as np
import ml_dtypes

import concourse.bass as bass
import concourse.mybir as mybir
import concourse.tile as tile
from concourse import bacc
from concourse.bass_utils import run_bass_kernel_spmd
from concourse.masks import make_identity

B, D = 1024, 512
NCORES = 8
NTILE = 512  # dataset rows per n-tile
NBT = B // 128  # 8 b-tiles

F32 = mybir.dt.float32
F32R = mybir.dt.float32r
BF16 = mybir.dt.bfloat16

AF = mybir.ActivationFunctionType
OP = mybir.AluOpType
AX = mybir.AxisListType

SIM_1CORE = False  # build single-core, no collectives (for TimelineSim)
LINEARIZE = False
ROUND_ENGINE = "pool"  # engine for fp32 -> f32r rounding copies
PROBT_ENGINE = "dve"  # engine for probT psum -> sbuf copy
ACC_ENGINE = "act"  # engine for acc psum -> sbuf copy
USE_XBAR = False  # DMA-transpose for probT (else PE transposes + copy)
BUFS_DT = 2
BUFS_NAT = 3
BUFS_SF = 6
BUFS_TINY = 5
BUFS_PSL = 3
BUFS_PSA = 2
BUFS_PSP = 2


def _copy(nc, eng, dst, src):
    if eng == "pool":
        nc.gpsimd.tensor_copy(dst, src)
    elif eng == "dve":
        nc.vector.tensor_copy(dst, src)
    else:
        nc.scalar.copy(dst, src)


def build(n_tiles):
    n_sh = n_tiles * NTILE
    ndev = 1 if SIM_1CORE else NCORES
    nc = bacc.Bacc("TRN2", target_bir_lowering=False, debug=False, num_devices=ndev)

    dT_p = nc.declare_dram_parameter("dT", [D, n_sh], F32, isOutput=False)
    natb_p = nc.declare_dram_parameter("natb", [n_sh, D], BF16, isOutput=False)
    dn6_p = nc.declare_dram_parameter("dn6", [6, n_sh], BF16, isOutput=False)
    w6_p = nc.declare_dram_parameter("w6", [6, B], BF16, isOutput=False)
    xT_p = nc.declare_dram_parameter("xT", [D, B], F32, isOutput=False)
    xrow_p = nc.declare_dram_parameter("xrow", [B, D], F32, isOutput=False)
    # per-b coefficient vectors, column layout [128, 8]: col i holds b = i*128+p
    ucol_p = nc.declare_dram_parameter("ucol", [128, NBT], F32, isOutput=False)
    nucol_p = nc.declare_dram_parameter("nucol", [128, NBT], F32, isOutput=False)
    dcol_p = nc.declare_dram_parameter("dcol", [128, NBT], F32, isOutput=False)
    vcol_p = nc.declare_dram_parameter("vcol", [128, NBT], F32, isOutput=False)
    out = nc.declare_dram_parameter("out", [B, D], F32, isOutput=True)

    dT_t = dT_p.ap().rearrange("(k p) (t n) -> t p k n", p=128, n=NTILE)
    natb_t = natb_p.ap().rearrange("(t j p) d -> t p j d", j=4, p=128)
    dn6_t = dn6_p.ap().rearrange("r (t n) -> t r n", n=NTILE)
    xT_t = xT_p.ap().rearrange("(k p) b -> k p b", p=128)  # [4, 128, B]
    xrow_t = xrow_p.ap().rearrange("(i p) d -> i p d", p=128)
    out_t = out.ap().rearrange("(i p) d -> i p d", p=128)

    with tile.TileContext(nc, linearize=LINEARIZE) as tc:
        with (
            tc.tile_pool(name="persist", bufs=1) as pp,
            tc.tile_pool(name="xf", bufs=2) as xfp,
            tc.tile_pool(name="dtf", bufs=BUFS_DT) as dtfp,
            tc.tile_pool(name="dtr", bufs=BUFS_DT) as dtrp,
            tc.tile_pool(name="nat", bufs=BUFS_NAT) as natp,
            tc.tile_pool(name="dn", bufs=BUFS_NAT) as dnp,
            tc.tile_pool(name="sf", bufs=BUFS_SF) as sfp,
            tc.tile_pool(name="tiny", bufs=BUFS_TINY) as tp,
            tc.tile_pool(name="fin", bufs=2) as finp,
            tc.tile_pool(name="psL", bufs=BUFS_PSL, space="PSUM") as psL,
            tc.tile_pool(name="psA", bufs=BUFS_PSA, space="PSUM") as psA,
            tc.tile_pool(name="psP", bufs=BUFS_PSP, space="PSUM") as psP,
            tc.tile_pool(name="dram", bufs=1, space="DRAM") as dram,
        ):
            # ---------------- constants / setup ----------------
            ident = pp.tile([128, 128], F32)
            make_identity(nc, ident[:])
            ident_bf = pp.tile([128, 128], BF16)
            nc.vector.tensor_copy(ident_bf[:], ident[:])

            ucol = pp.tile([128, NBT], F32)
            nucol = pp.tile([128, NBT], F32)
            dcol = pp.tile([128, NBT], F32)
            vcol = pp.tile([128, NBT], F32)
            for t_, p_ in ((ucol, ucol_p), (nucol, nucol_p), (dcol, dcol_p), (vcol, vcol_p)):
                nc.sync.dma_start(out=t_[:], in_=p_.ap())

            w6 = pp.tile([6, B], BF16)
            nc.sync.dma_start(out=w6[:], in_=w6_p.ap())

            # xT chunks, rounded to f32r once
            xr = [pp.tile([128, B], F32R, tag=f"xr{k}", name=f"xr{k}") for k in range(4)]
            for k in range(4):
                xf = xfp.tile([128, B], F32, tag="xf")
                nc.sync.dma_start(out=xf[:], in_=xT_t[k])
                nc.vector.tensor_copy(xr[k][:], xf[:])

            # running stats
            m_run = pp.tile([128, NBT], F32)
            l_run = pp.tile([128, NBT], F32)
            acc = [pp.tile([128, D], F32, tag=f"acc{i}", name=f"acc{i}") for i in range(NBT)]
            nc.vector.memset(m_run[:], -1.0e30)
            nc.vector.memset(l_run[:], 0.0)
            for i in range(NBT):
                nc.vector.memset(acc[i][:], 0.0)

            # ---------------- main loop over dataset tiles ----------------
            # Software-pipelined: MM1 of b-tile i is emitted before the
            # softmax tail of b-tile i-1, so PE overlaps the DVE/ACT chain.

            def emit_mm1(i, dTr_all, dn6t):
                bi = slice(i * 128, (i + 1) * 128)
                pL = psL.tile([128, NTILE], F32, tag="pL")
                for k in range(4):
                    nc.tensor.matmul(
                        pL[:], xr[k][:, bi], dTr_all[:, k * NTILE:(k + 1) * NTILE],
                        start=(k == 0), stop=False,
                    )
                nc.tensor.matmul(pL[:], w6[:, bi], dn6t[:], start=False, stop=True)
                return pL

            def emit_chain(i, pL):
                # online max update
                mt = tp.tile([128, 1], F32, tag="mt")
                nc.vector.tensor_reduce(mt[:], pL[:], axis=AX.X, op=OP.max)
                dlt = tp.tile([128, 1], F32, tag="dlt")
                # dlt = min(m_old - mt, 0) = m_old - m_new
                nc.vector.tensor_scalar(
                    out=dlt[:], in0=m_run[:, i:i + 1], scalar1=mt[:],
                    scalar2=0.0, op0=OP.subtract, op1=OP.min,
                )
                nc.vector.tensor_tensor(
                    m_run[:, i:i + 1], m_run[:, i:i + 1], mt[:], op=OP.max
                )
                alpha = tp.tile([128, 1], F32, tag="alpha")
                nc.scalar.activation(
                    alpha[:], dlt[:], AF.Exp, bias=0.0, scale=ucol[:, i:i + 1]
                )
                # bias = -u * m_new
                ebias = tp.tile([128, 1], F32, tag="ebias")
                nc.vector.tensor_tensor(
                    ebias[:], nucol[:, i:i + 1], m_run[:, i:i + 1], op=OP.mult
                )
                # prob = exp(u*P + bias), lt = rowsum
                prob = sfp.tile([128, NTILE], BF16, tag="prob")
                lt = tp.tile([128, 1], F32, tag="lt")
                nc.scalar.activation(
                    prob[:], pL[:], AF.Exp,
                    bias=ebias[:], scale=ucol[:, i:i + 1], accum_out=lt[:],
                )
                # l = l*alpha + lt (fused)
                nc.vector.scalar_tensor_tensor(
                    out=l_run[:, i:i + 1], in0=l_run[:, i:i + 1],
                    scalar=alpha[:], in1=lt[:], op0=OP.mult, op1=OP.add,
                )
                # probT transpose (bf16): xbar DMA or PE + copy
                probT = sfp.tile([128, NTILE], BF16, tag="probT")
                if USE_XBAR:
                    for k in range(0, NTILE, 128):
                        ksl = slice(k, k + 128)
                        nc.sync.dma_start_transpose(probT[:, ksl], prob[:, ksl])
                else:
                    pP = psP.tile([128, NTILE], BF16, tag="pP", name="pP")
                    for k in range(0, NTILE, 128):
                        ksl = slice(k, k + 128)
                        nc.tensor.transpose(pP[:, ksl], prob[:, ksl], ident_bf[:])
                    _copy(nc, PROBT_ENGINE, probT[:], pP[:])
                return probT, alpha

            def emit_mm2(i, probT, alpha, natbf_all):
                # MM2: pA = probT-chunks @ natbf; acc = alpha*acc + pA (DVE)
                pA = psA.tile([128, D], F32, tag="pA")
                for k in range(4):
                    ksl = slice(k * 128, (k + 1) * 128)
                    nc.tensor.matmul(
                        pA[:], probT[:, ksl],
                        natbf_all[:, k * D:(k + 1) * D],
                        start=(k == 0), stop=(k == 3),
                    )
                nc.vector.scalar_tensor_tensor(
                    out=acc[i][:], in0=acc[i][:],
                    scalar=alpha[:], in1=pA[:], op0=OP.mult, op1=OP.add,
                )

            def emit_loads(t):
                natbf_all = natp.tile([128, 4 * D], BF16, tag="natbf")
                dTf_all = dtfp.tile([128, 4 * NTILE], F32, tag="dTf")
                dTr_all = dtrp.tile([128, 4 * NTILE], F32R, tag="dTr")
                nc.sync.dma_start(
                    out=natbf_all[:].rearrange("p (j d) -> p j d", j=4),
                    in_=natb_t[t],
                )
                nc.sync.dma_start(
                    out=dTf_all[:].rearrange("p (k n) -> p k n", k=4),
                    in_=dT_t[t],
                )
                _copy(nc, ROUND_ENGINE, dTr_all[:], dTf_all[:])
                dn6t = dnp.tile([6, NTILE], BF16, tag="dn6t")
                nc.sync.dma_start(out=dn6t[:], in_=dn6_t[t])
                return natbf_all, dTr_all, dn6t

            # depth-2 pipeline: iteration emits MM1(j), MM2(j-2), chain(j-1);
            # the tile's natbf travels with each queued item.
            nxt = emit_loads(0)
            chain_q = []  # (i, pL, natbf_all)
            mm2_q = []  # (i, probT, diag, natbf_all)

            def step_queues(drain=False):
                if len(mm2_q) >= (1 if drain else 2):
                    emit_mm2(*mm2_q.pop(0))
                if chain_q:
                    ci, cpL, cnat = chain_q.pop(0)
                    probT, diag = emit_chain(ci, cpL)
                    mm2_q.append((ci, probT, diag, cnat))

            for t in range(n_tiles):
                cur = nxt
                for i in range(NBT):
                    pL = emit_mm1(i, cur[1], cur[2])
                    step_queues()
                    chain_q.append((i, pL, cur[0]))
                    if i == 1 and t + 1 < n_tiles:
                        nxt = emit_loads(t + 1)
            while chain_q or mm2_q:
                step_queues(drain=True)

            # ---------------- cross-core merge ----------------
            m_cc_in = dram.tile([128, NBT], F32)
            m_cc_out = dram.tile([128, NBT], F32)
            nc.sync.dma_start(out=m_cc_in[:], in_=m_run[:])
            if not SIM_1CORE:
                nc.gpsimd.collective_compute(
                    "AllReduce", OP.max,
                    replica_groups=[list(range(NCORES))],
                    ins=[m_cc_in[:].opt()], outs=[m_cc_out[:].opt()],
                )
            else:
                nc.sync.dma_start(out=m_cc_out[:], in_=m_cc_in[:])
            m_glob = pp.tile([128, NBT], F32)
            nc.sync.dma_start(out=m_glob[:], in_=m_cc_out[:])

            # gamma_i = exp(u * (m_loc - m_glob)); scale acc, l
            dg = pp.tile([128, NBT], F32)
            nc.vector.tensor_tensor(dg[:], m_run[:], m_glob[:], op=OP.subtract)
            gam = pp.tile([128, NBT], F32)
            for i in range(NBT):
                nc.scalar.activation(
                    gam[:, i:i + 1], dg[:, i:i + 1], AF.Exp,
                    bias=0.0, scale=ucol[:, i:i + 1],
                )
            nc.vector.tensor_tensor(l_run[:], l_run[:], gam[:], op=OP.mult)

            accl_in = dram.tile([128, NBT * D + NBT], F32)
            accl_out = dram.tile([128, NBT * D + NBT], F32)
            for i in range(NBT):
                accs = finp.tile([128, D], F32, tag="accs")
                nc.vector.tensor_scalar(
                    out=accs[:], in0=acc[i][:], scalar1=gam[:, i:i + 1],
                    scalar2=None, op0=OP.mult,
                )
                nc.sync.dma_start(out=accl_in[:, i * D:(i + 1) * D], in_=accs[:])
            nc.sync.dma_start(out=accl_in[:, NBT * D:], in_=l_run[:])
            if not SIM_1CORE:
                nc.gpsimd.collective_compute(
                    "AllReduce", OP.add,
                    replica_groups=[list(range(NCORES))],
                    ins=[accl_in[:].opt()], outs=[accl_out[:].opt()],
                )
            else:
                nc.sync.dma_start(out=accl_out[:], in_=accl_in[:])

            lg = pp.tile([128, NBT], F32)
            nc.sync.dma_start(out=lg[:], in_=accl_out[:, NBT * D:])
            rl = pp.tile([128, NBT], F32)
            nc.vector.reciprocal(rl[:], lg[:])
            # s1 = dcoef / l
            s1 = pp.tile([128, NBT], F32)
            nc.vector.tensor_tensor(s1[:], dcol[:], rl[:], op=OP.mult)
            for i in range(NBT):
                accg = finp.tile([128, D], F32, tag="accg")
                nc.sync.dma_start(out=accg[:], in_=accl_out[:, i * D:(i + 1) * D])
                xnat = xfp.tile([128, D], F32, tag="xnat")
                nc.sync.dma_start(out=xnat[:], in_=xrow_t[i])
                v1 = finp.tile([128, D], F32, tag="v1")
                nc.vector.tensor_scalar(
                    out=v1[:], in0=accg[:], scalar1=s1[:, i:i + 1],
                    scalar2=None, op0=OP.mult,
                )
                v2 = finp.tile([128, D], F32, tag="v2")
                nc.vector.tensor_scalar(
                    out=v2[:], in0=xnat[:], scalar1=vcol[:, i:i + 1],
                    scalar2=None, op0=OP.mult,
                )
                nc.vector.tensor_tensor(v1[:], v1[:], v2[:], op=OP.add)
                nc.sync.dma_start(out=out_t[i], in_=v1[:])

    nc.compile()
    return nc


_BUILD_CACHE = {}


def _get_nc(n_tiles):
    key = (n_tiles, SIM_1CORE, LINEARIZE, ROUND_ENGINE, PROBT_ENGINE, ACC_ENGINE, USE_XBAR,
           BUFS_DT, BUFS_NAT, BUFS_SF, BUFS_TINY, BUFS_PSL, BUFS_PSA, BUFS_PSP)
    if key not in _BUILD_CACHE:
        _BUILD_CACHE[key] = build(n_tiles)
    return _BUILD_CACHE[key]


def _bf16(x):
    """Fast round-to-nearest-even fp32 -> bf16 (returns ml_dtypes.bfloat16)."""
    u = np.ascontiguousarray(x, dtype=np.float32).view(np.uint32)
    r = (u + np.uint32(0x7FFF) + ((u >> np.uint32(16)) & np.uint32(1))) >> np.uint32(16)
    return r.astype(np.uint16).view(ml_dtypes.bfloat16)


def _split3_bf16(v):
    """3-way bf16 split of a float64/float32 vector: v ~ s1+s2+s3."""
    v = v.astype(np.float32)
    s1 = _bf16(v)
    r1 = v - s1.astype(np.float32)
    s2 = _bf16(r1)
    r2 = r1 - s2.astype(np.float32)
    s3 = _bf16(r2)
    return s1, s2, s3


def make_in_maps(x_t, t, dataset, n_tiles):
    """Shard + pad dataset, precompute transposes/casts/norms + coeffs."""
    n = dataset.shape[0]
    n_sh = n_tiles * NTILE
    n_pad = NCORES * n_sh
    assert n_pad >= n
    dpad = np.zeros((n_pad, D), dtype=np.float32)
    dpad[:n] = dataset
    dpad[n:, 0] = 1000.0  # far-away pad rows: huge norm, ~zero softmax weight

    a = t.astype(np.float64)
    b = 1.0 - a
    u = (a / (b * b)).astype(np.float32)
    w = -a / 2.0
    dcoef = (1.0 + a / b).astype(np.float32)
    vcoef = (-1.0 / b).astype(np.float32)

    w1, w2, w3 = _split3_bf16(w)
    w6 = np.stack([w1, w1, w1, w2, w2, w3])  # [6, B] bf16

    dn = np.einsum(
        "nd,nd->n", dpad.astype(np.float64), dpad.astype(np.float64)
    )
    d1, d2, d3 = _split3_bf16(dn)
    dn6 = np.stack([d1, d2, d3, d1, d2, d1])  # [6, n_pad] bf16

    dT = np.ascontiguousarray(dpad.T)  # [D, n_pad] fp32
    natb = _bf16(dpad).reshape(n_pad, D)

    def col(v):
        return np.ascontiguousarray(v.reshape(NBT, 128).T)

    base = dict(
        xT=np.ascontiguousarray(x_t.T),
        xrow=np.ascontiguousarray(x_t),
        w6=np.ascontiguousarray(w6),
        ucol=col(u),
        nucol=col(-u),
        dcol=col(dcoef),
        vcol=col(vcoef),
    )
    return [
        dict(
            base,
            dT=np.ascontiguousarray(dT[:, c * n_sh:(c + 1) * n_sh]),
            natb=np.ascontiguousarray(natb[c * n_sh:(c + 1) * n_sh]),
            dn6=np.ascontiguousarray(dn6[:, c * n_sh:(c + 1) * n_sh]),
        )
        for c in range(NCORES)
    ]


def kernel(x_t, t, dataset):
    x_t = np.asarray(x_t, dtype=np.float32)
    t = np.asarray(t, dtype=np.float32)
    dataset = np.asarray(dataset, dtype=np.float32)
    n = dataset.shape[0]
    n_tiles = -(-n // (NCORES * NTILE))  # ceil -> 25 for N=100000
    nc = _get_nc(n_tiles)
    in_maps = make_in_maps(x_t, t, dataset, n_tiles)
    res = run_bass_kernel_spmd(nc, in_maps, core_ids=list(range(NCORES)))
    return np.asarray(res.results[0]["out"], dtype=np.float32)


def ref_numpy(x_t, t, dataset):
    aa = t.astype(np.float64)
    bb = 1.0 - aa
    dsn = (dataset.astype(np.float64) ** 2).sum(1)
    t2 = x_t.astype(np.float64) @ dataset.T.astype(np.float64)
    logit = (-1.0 / (2 * bb * bb))[:, None] * (
        (x_t.astype(np.float64) ** 2).sum(1)[:, None]
        - 2 * aa[:, None] * t2
        + (aa * aa)[:, None] * dsn[None, :]
    )
    p = np.exp(logit - logit.max(1, keepdims=True))
    p /= p.sum(1, keepdims=True)
    p = p * (1 + aa / bb)[:, None]
    return (-1.0 / bb)[:, None] * x_t.astype(np.float64) + p @ dataset.astype(np.float64)


if __name__ == "__main__":
    rng = np.random.default_rng(0)
    n = 2 * NCORES * NTILE - 300
    x_t = rng.standard_normal((B, D)).astype(np.float32)
    t = rng.uniform(0.05, 0.95, (B,)).astype(np.float32)
    dataset = rng.standard_normal((n, D)).astype(np.float32)
    v = kernel(x_t, t, dataset)
    vref = ref_numpy(x_t, t, dataset)
    err = np.linalg.norm(v - vref) / np.linalg.norm(vref)
    print("rel l2 err:", err)
    print("max abs err:", np.abs(v - vref).max(), "ref scale:", np.abs(vref).max())
